# revision 21
# baseline (speedup 1.0000x reference)
"""Trainium2 Bass kernel for nn_DyConv (MoE routed dynamic conv).

Model (per batch image b):
  g = mean(x[b], spatial)                      # [C]
  w = softmax(fc2(relu(fc1(g))))               # [E]  router weights
  out[b] = sum_e w[e] * silu(bn_e(conv3x3_e(x[b])))

Strategy: pure data-parallel over batch. B=16 images / 8 cores = 2 images
per core; router + experts replicated. No collectives.

Per-core device program (per image):
  - x arrives host-padded to a flat 162x162 zero-padded layout in bf16.
  - Two SBUF "region" copies per half-image stack 2 shifted taps on the
    128 partitions: A = [x(+0); x(+1)], B = [x(+2); x(+164)].  A K=128
    matmul against A at offset o contracts taps (o, o+1) for all 64
    channels; 9 conv taps = 5 matmuls (3xA-pair, 1xB-pair, 1 half-K
    single) per expert-pair.  Two expert pairs stacked in M=128.
  - BN scale folded into conv weights on host; BN shift applied as the
    per-partition bias of the SiLU activation (ScalarE, PSUM->SBUF bf16).
  - Router: DVE chunk-reduces over the bf16 image; tiny matmuls + a
    tanh-based exp for the 4-way softmax; the mixing weights become two
    scaled-identity bf16 [128,64] lhsT tiles built on ScalarE.  The
    router is emitted in 4 stages one conv-tile apart, and image s+1's
    router is computed in the middle of image s.
  - Mix: 2 accumulating matmuls -> PSUM [64,N]; DVE compacting copy
    (drops the 2 pad cols) into a 3-tile (9-row) bounce group; one
    contiguous DMA per group to the output.
  - The TileScheduler reorders aggressively, so every order-sensitive
    instruction (DVE stream, DMA dispatch chains per queue, router
    stage ops) carries explicit sync=False ordering edges pinning it
    to its intended queue position.
"""
import os
import sys
import numpy as np

if "/opt/trn_rl_repo" not in sys.path:
    sys.path.insert(0, "/opt/trn_rl_repo")

import ml_dtypes  # noqa: E402

BF16_NP = ml_dtypes.bfloat16
FP8_NP = ml_dtypes.float8_e4m3fn

B, C, H, W = 16, 64, 160, 160
E, R = 4, 16
NCORES = 8
IMG_PER_CORE = B // NCORES          # 2
WP = W + 2                          # 162 padded row
LP = (H + 2) * WP                   # 26244 padded flat image
LHOST = 26600                       # host buffer with zero margin (max read 26408)
RLEN = 82 * WP                      # 13284: half-image region (80 out rows + 2 halo)
HB = 80 * WP                        # 12960: out-grid columns per half
NT = 486                            # psum tile = 3 out rows
BN_EPS = 1e-3

# chunk boundaries within a half-image region
CH5 = [0, 830, 1661, 3321, 6642, 9963, 13284]  # first-needed regions: small head
CH3 = [0, 830, 1661, 6642, 13284]
CH2 = [0, 6642, 13284]                      # steady-state prefetch loads
# router tile: half-image split across the partition dim so the DVE
# reduce covers the half in RTC columns; rows 0:64 and 64:128 are summed
# for free by the K=128 router matmul (fc1^T stacked twice).
RTC = 6642                                  # rt tile columns (h1 half size)
RTH = {0: 6480, 1: 6642}                    # per-half split point (h0: 80 rows)

_CACHE = {}


def _build_program(reps=1):
    import concourse.bacc as bacc
    import concourse.tile as tile
    from concourse import mybir
    from concourse.tile_rust import add_dep_helper

    BF16 = mybir.dt.bfloat16
    F32 = mybir.dt.float32
    AF = mybir.ActivationFunctionType
    ALU = mybir.AluOpType
    AX = mybir.AxisListType

    nc = bacc.Bacc("TRN2", target_bir_lowering=False, debug=False,
                   num_devices=NCORES)

    FP8 = mybir.dt.float8e4
    xp_d = nc.dram_tensor("xp", [IMG_PER_CORE, 4, C, LHOST], BF16, kind="ExternalInput")
    xq_d = nc.dram_tensor("xq", [IMG_PER_CORE, C, LHOST], FP8, kind="ExternalInput")
    cbf_d = nc.dram_tensor("cbf", [128, 1300], BF16, kind="ExternalInput")
    cf32_d = nc.dram_tensor("cf32", [128, 131], F32, kind="ExternalInput")
    out_d = nc.dram_tensor("out", [IMG_PER_CORE, C, H, W], F32, kind="ExternalOutput")

    seq = [i % IMG_PER_CORE for i in range(IMG_PER_CORE * reps)]
    S = len(seq)

    with tile.TileContext(nc) as tc:
        with tc.tile_pool(name="consts", bufs=1) as cp, \
             tc.tile_pool(name="regs", bufs=2) as rp, \
             tc.tile_pool(name="work", bufs=2) as wp, \
             tc.tile_pool(name="bounce", bufs=3) as bp, \
             tc.tile_pool(name="psum", bufs=1, space="PSUM") as pp:

            cbf = cp.tile([128, 1300], BF16)
            cb1 = nc.sync.dma_start(cbf[:, 0:640], cbf_d[:, 0:640])
            cb2 = nc.sync.dma_start(cbf[:, 640:1300], cbf_d[:, 640:1300])
            add_dep_helper(cb2.ins, cb1.ins, sync=False, reason="order-pin")
            cf32 = cp.tile([128, 131], F32)
            last_cf = nc.gpsimd.dma_start(cf32[:, :], cf32_d[:, :])
            o64_sb = cp.tile([1, 64], F32)
            nc.gpsimd.memset(o64_sb[:, :], 1.0)

            fc1t_sb = cbf[0:128, 1280:1296]
            fc2t_sb = cbf[0:17, 1296:1300]
            idc_sb = cf32[:, 0:128]
            bnb_sb = cf32[:, 128:130]
            fc2b_sb = cf32[0:4, 130:131]

            regA_t = {}
            regB_t = {}
            parts_t = {}
            mixw_of = {}
            routst = {}
            # ordering-pin state: last instruction per in-order stream
            last = {"dve": None, "gp": last_cf, "sc": None, "syb": None,
                    "out": None, "conv": None, "silu": None}

            def dep(a, b):
                if a is not None and b is not None:
                    add_dep_helper(a.ins, b.ins, sync=False, reason="order-pin")

            def dve(inst):
                dep(inst, last["dve"])
                last["dve"] = inst
                return inst

            def disp_region(s, h, reg, qkey, chb, extra_anchor=None):
                img = seq[s]
                if reg == "A":
                    if (s, h) not in regA_t:
                        regA_t[(s, h)] = rp.tile([128, RLEN], BF16, tag="regA",
                                                 name=f"regA{s}h{h}", bufs=3)
                    t = regA_t[(s, h)]
                    j0 = 0
                else:
                    if (s, h) not in regB_t:
                        regB_t[(s, h)] = rp.tile([128, RLEN], BF16, tag="regB",
                                                 name=f"regB{s}h{h}", bufs=2)
                    t = regB_t[(s, h)]
                    j0 = 2
                eng = {"gp": nc.gpsimd, "sc": nc.scalar, "syb": nc.sync}[qkey]
                for c in range(len(chb) - 1):
                    i = eng.dma_start(
                        t[:, chb[c]:chb[c + 1]],
                        xp_d[img, j0:j0 + 2, :,
                             h * HB + chb[c]:h * HB + chb[c + 1]]
                        .rearrange("j c f -> (j c) f"))
                    dep(i, last[qkey])
                    dep(i, extra_anchor)
                    last[qkey] = i

            rt_t = {}

            def disp_rt(s, h, qkey, extra_anchor=None):
                # fp8 router tile: rows 0:64 = first RTH[h] cols of the
                # half, rows 64:128 = the rest; fc1^T rows stacked 2x so
                # the K=128 matmul sums both halves for free
                img = seq[s]
                t = rp.tile([128, RTC], FP8, tag="rt", name=f"rt{s}h{h}",
                            bufs=2)
                rt_t[(s, h)] = t
                L = RTH[h]
                eng = {"gp": nc.gpsimd, "sc": nc.scalar, "syb": nc.sync}[qkey]
                for r0, off in ((0, 0), (64, L)):
                    i = eng.dma_start(
                        t[r0:r0 + 64, 0:L],
                        xq_d[img, :, h * HB + off:h * HB + off + L])
                    dep(i, last[qkey])
                    dep(i, extra_anchor)
                    last[qkey] = i

            def emit_reduce_chunk(s, h, c):
                # partial router sum over one rt column chunk (DVE, 128 rows)
                if s not in parts_t:
                    parts_t[s] = wp.tile([128, 4], F32, tag="parts", bufs=2,
                                         name=f"parts{s}")
                L = RTH[h]
                a, b = c * L // 2, (c + 1) * L // 2
                dve(nc.vector.tensor_reduce(
                    parts_t[s][:, h * 2 + c:h * 2 + c + 1],
                    rt_t[(s, h)][:, a:b],
                    axis=AX.X, op=ALU.add))

            def emit_router_stage(s, k):
                st = routst.setdefault(s, {})
                pconv, psilu = last["conv"], last["silu"]
                if k == 0:
                    st["gsum"] = wp.tile([128, 1], F32, tag="gsum", name=f"gsum{s}")
                    dve(nc.vector.tensor_reduce(st["gsum"][:, :], parts_t[s][:, 0:4],
                                                axis=AX.X, op=ALU.add))
                    st["gbf"] = wp.tile([128, 1], BF16, tag="gbf", name=f"gbf{s}")
                    dve(nc.vector.tensor_copy(st["gbf"][:, :], st["gsum"][:, :]))
                    h_ps = pp.tile([16, 1], F32, tag="pr", name=f"hps{s}")
                    dep(nc.tensor.matmul(h_ps[:, :], fc1t_sb, st["gbf"][:, :],
                                         start=True, stop=True), pconv)
                    st["hbf"] = wp.tile([32, 1], BF16, tag="hbf", name=f"hbf{s}")
                    dve(nc.vector.memset(st["hbf"][:, :], 1.0))
                    dep(nc.scalar.activation(st["hbf"][0:16, :], h_ps[:, :], AF.Relu),
                        psilu)
                elif k == 1:
                    # logits as a row: [hbf;1]^T @ [fc2^T; fc2b^T]  (K=17)
                    lrow_ps = pp.tile([1, 4], F32, tag="pr", name=f"lrow{s}")
                    dep(nc.tensor.matmul(lrow_ps[:, :], st["hbf"][0:17, :],
                                         fc2t_sb, start=True, stop=True), pconv)
                    st["lrow"] = lrow_ps
                elif k == 2:
                    # exp(l) = (1 + tanh(l/2)) / (1 - tanh(l/2)); logits are O(0.5)
                    trow = wp.tile([1, 4], F32, tag="trow", name=f"trow{s}")
                    dep(nc.scalar.activation(trow[:, :], st["lrow"][:, :], AF.Tanh,
                                             scale=0.5), psilu)
                    num = wp.tile([1, 4], F32, tag="num", name=f"num{s}")
                    dve(nc.vector.tensor_scalar_add(num[:, :], trow[:, :], 1.0))
                    den = wp.tile([1, 4], F32, tag="den", name=f"den{s}")
                    dve(nc.vector.tensor_scalar(den[:, :], trow[:, :], -1.0, 1.0,
                                                op0=ALU.mult, op1=ALU.add))
                    rec = wp.tile([1, 4], F32, tag="rec", name=f"rec{s}")
                    dve(nc.vector.reciprocal(rec[:, :], den[:, :]))
                    erow = wp.tile([1, 4], F32, tag="erow", name=f"erow{s}")
                    dve(nc.vector.tensor_tensor(erow[:, :], num[:, :], rec[:, :],
                                                op=ALU.mult))
                    ssum = wp.tile([1, 1], F32, tag="ssum", name=f"ssum{s}")
                    dve(nc.vector.tensor_reduce(ssum[:, :], erow[:, :], axis=AX.X,
                                                op=ALU.add))
                    sinv = wp.tile([1, 1], F32, tag="sinv", name=f"sinv{s}")
                    dve(nc.vector.reciprocal(sinv[:, :], ssum[:, :]))
                    wrow = wp.tile([1, 4], F32, tag="wrow", name=f"wrow{s}")
                    dve(nc.vector.tensor_scalar_mul(wrow[:, :], erow[:, :],
                                                    sinv[:, 0:1]))
                    rowA = wp.tile([1, 128], F32, tag="rowA", name=f"rowA{s}")
                    dve(nc.vector.tensor_scalar_mul(rowA[:, 0:64], o64_sb[:, :],
                                                    wrow[:, 0:1]))
                    dve(nc.vector.tensor_scalar_mul(rowA[:, 64:128], o64_sb[:, :],
                                                    wrow[:, 1:2]))
                    rowB = wp.tile([1, 128], F32, tag="rowB", name=f"rowB{s}")
                    dve(nc.vector.tensor_scalar_mul(rowB[:, 0:64], o64_sb[:, :],
                                                    wrow[:, 2:3]))
                    dve(nc.vector.tensor_scalar_mul(rowB[:, 64:128], o64_sb[:, :],
                                                    wrow[:, 3:4]))
                    st["rowA"], st["rowB"] = rowA, rowB
                else:
                    wc_ps = pp.tile([128, 2], F32, tag="pr", name=f"wc{s}")
                    dep(nc.tensor.matmul(wc_ps[:, 0:1], st["rowA"][:, :],
                                         idc_sb[0:1, 0:1], start=True, stop=True),
                        pconv)
                    dep(nc.tensor.matmul(wc_ps[:, 1:2], st["rowB"][:, :],
                                         idc_sb[0:1, 0:1], start=True, stop=True),
                        pconv)
                    wcol = wp.tile([128, 2], F32, tag="wcol", name=f"wcol{s}")
                    dep(nc.scalar.copy(wcol[:, :], wc_ps[:, :]), psilu)
                    mA = wp.tile([128, 128], BF16, tag="mixA", name=f"mixA{s}")
                    dep(nc.scalar.activation(mA[:, :], idc_sb, AF.Copy,
                                             scale=wcol[:, 0:1]), psilu)
                    mB = wp.tile([128, 128], BF16, tag="mixB", name=f"mixB{s}")
                    dep(nc.scalar.activation(mB[:, :], idc_sb, AF.Copy,
                                             scale=wcol[:, 1:2]), psilu)
                    mixw_of[s] = (mA, mB)

            # ---- mix + compacting copy + grouped output DMA ----
            grp_state = {}

            def emit_mix(st):
                tsbs, N, h, t, s = st
                img = seq[s]
                mA, mB = mixw_of[s]
                po = pp.tile([128, N], F32, tag="po", bufs=3)
                nc.tensor.matmul(po[:, :], mA[:, :], tsbs[0][:, :],
                                 start=True, stop=False)
                nc.tensor.matmul(po[:, :], mB[:, :], tsbs[1][:, :],
                                 start=False, stop=True)
                g, j = divmod(t, 3)
                if j == 0:
                    grp_state["t"] = bp.tile([64, 1440], F32, tag="bounce",
                                             name="bounce", bufs=3)
                bt = grp_state["t"]
                nrows = 3 if t < 26 else 2
                src = po[0:64, :].rearrange("p (r c) -> p r c", c=WP)[:, 0:nrows, 0:W]
                dst = bt[:, j * 480:j * 480 + nrows * W].rearrange(
                    "p (r c) -> p r c", c=W)
                dve(nc.vector.tensor_copy(dst, src))
                if s == S - 1 and h == 1 and t >= 24:
                    r0 = h * 80 + t * 3
                    i = nc.sync.dma_start(
                        out_d[img, :, r0:r0 + nrows, :],
                        bt[:, j * 480:j * 480 + nrows * W].rearrange(
                            "p (r c) -> p r c", c=W))
                    dep(i, last["out"])
                    last["out"] = i
                elif j == 2 or t == 26:
                    r0 = h * 80 + g * 9
                    nr = 9 if g < 8 else 8
                    i = nc.sync.dma_start(
                        out_d[img, :, r0:r0 + nr, :],
                        bt[:, 0:nr * W].rearrange("p (r c) -> p r c", c=W))
                    dep(i, last["out"])
                    last["out"] = i

            # ---- main loop over image sequence ----
            pending = []
            for s in range(S):
                if s == 0:
                    # <=8 dispatches per shared sem pool at startup:
                    # gpsimd: cf32,A00x5,A01x2; scalar: B00x3; sync:
                    # cbf,rt00x2,rt01x2.  B01 follows at gtile 6.
                    disp_region(0, 0, "A", "gp", CH5[0:5])
                    disp_region(0, 1, "A", "gp", CH2)
                    disp_region(0, 0, "B", "sc", CH3)
                else:
                    # h1's B region: its buffer frees at image s-1's end
                    disp_region(s, 1, "B", "syb", CH2, extra_anchor=last["out"])
                if s + 1 < S:
                    disp_region(s + 1, 0, "A", "gp", CH2)
                ROUTER_G = 9 if s == 0 else -10  # stages 9-12, mixes from 13
                for h in (0, 1):
                    regA = regA_t[(s, h)]
                    regB = regB_t[(s, h)]
                    for t in range(27):
                        gtile = h * 27 + t
                        if s == 0:
                            if gtile == 2:
                                disp_rt(0, 0, "syb")
                            elif gtile == 3:
                                emit_reduce_chunk(0, 0, 0)
                                emit_reduce_chunk(0, 0, 1)
                                disp_region(0, 0, "A", "syb", CH5[4:6])
                            elif gtile == 4:
                                disp_rt(0, 1, "syb")
                            elif gtile == 5:
                                emit_reduce_chunk(0, 1, 0)
                                emit_reduce_chunk(0, 1, 1)
                                disp_region(0, 0, "A", "syb", CH5[5:7])
                            elif gtile == 6:
                                disp_region(0, 1, "B", "sc", CH2,
                                            extra_anchor=last["silu"])
                        if s + 1 < S:
                            if gtile == 26:
                                disp_rt(s + 1, 0, "sc",
                                        extra_anchor=last["silu"])
                            elif gtile == 27:
                                disp_rt(s + 1, 1, "sc",
                                        extra_anchor=last["silu"])
                            elif gtile == 28:
                                disp_region(s + 1, 0, "B", "syb", CH2,
                                            extra_anchor=last["out"])
                            elif gtile == 30:
                                disp_region(s + 1, 1, "A", "sc", CH2,
                                            extra_anchor=last["silu"])
                            elif gtile in (38, 39):
                                emit_reduce_chunk(s + 1, 0, gtile - 38)
                            elif gtile in (44, 45):
                                emit_reduce_chunk(s + 1, 1, gtile - 44)
                            elif 48 <= gtile <= 51:
                                emit_router_stage(s + 1, gtile - 48)
                        N = NT if t < 26 else 324
                        n0 = t * NT
                        tsbs = []
                        for ep in range(2):
                            cps = pp.tile([128, N], F32, tag=f"pc{ep}", bufs=2)
                            base = ep * 640
                            nc.tensor.matmul(cps[:, :], cbf[:, base:base + 128],
                                             regA[:, n0:n0 + N], start=True, stop=False)
                            nc.tensor.matmul(cps[:, :], cbf[:, base + 128:base + 256],
                                             regA[:, n0 + 162:n0 + 162 + N],
                                             start=False, stop=False)
                            nc.tensor.matmul(cps[:, :], cbf[:, base + 256:base + 384],
                                             regA[:, n0 + 324:n0 + 324 + N],
                                             start=False, stop=False)
                            nc.tensor.matmul(cps[:, :], cbf[:, base + 384:base + 512],
                                             regB[:, n0:n0 + N], start=False, stop=False)
                            mlast = nc.tensor.matmul(
                                cps[:, :], cbf[:, base + 512:base + 640],
                                regB[:, n0 + 324:n0 + 324 + N],
                                start=False, stop=True)
                            tsb = wp.tile([128, N], BF16, tag=f"t{ep}", bufs=15)
                            slast = nc.scalar.activation(tsb[:, :], cps[:, :], AF.Silu,
                                                         bias=bnb_sb[:, ep:ep + 1])
                            tsbs.append(tsb)
                        last["conv"], last["silu"] = mlast, slast
                        pending.append((tsbs, N, h, t, s))
                        if 0 <= gtile - ROUTER_G <= 3:
                            emit_router_stage(s, gtile - ROUTER_G)
                        if s > 0 or gtile > ROUTER_G + 3:
                            k = 0
                            while len(pending) > 2 and k < 2:
                                emit_mix(pending.pop(0))
                                k += 1
                # drain this image's tail so bounce groups stay h-aligned
                while pending:
                    emit_mix(pending.pop(0))

    nc.compile()
    return nc


def _prep_weights(fc1_w, fc2_w, fc2_b, conv_w, bn_gamma, bn_beta, bn_mean, bn_var):
    scale = bn_gamma / np.sqrt(bn_var + BN_EPS)            # [E, C]
    shift = bn_beta - bn_mean * scale                      # [E, C]
    ws = conv_w * scale[:, :, None, None, None]            # [E, Co, Ci, 3, 3]

    # paired-tap lhsT blocks: [K=128 (2 taps x 64 ci), M=128 (2 experts x 64 co)]
    groups = [((0, 0), (0, 1)), ((1, 0), (1, 1)), ((2, 0), (2, 1)), ((0, 2), (1, 2))]
    wk = np.zeros((128, 1024), np.float32)
    for ep in range(2):
        for g, (ta, tb) in enumerate(groups):
            blk = np.stack([ws[:, :, :, ta[0], ta[1]], ws[:, :, :, tb[0], tb[1]]])
            blk = blk[:, 2 * ep:2 * ep + 2]                # [j, le, Co, Ci]
            lhsT = blk.transpose(0, 3, 1, 2).reshape(128, 128)
            wk[:, (ep * 4 + g) * 128:(ep * 4 + g + 1) * 128] = lhsT
    wks = np.zeros((128, 256), np.float32)                 # rows 64-127 stay zero
    s22 = ws[:, :, :, 2, 2]                                # [E, Co, Ci]
    for ep in range(2):
        blk = s22[2 * ep:2 * ep + 2]                       # [le, Co, Ci]
        wks[0:64, ep * 128:(ep + 1) * 128] = blk.transpose(2, 0, 1).reshape(64, 128)

    # packed bf16 consts: per-ep [wk 512 | wks 128] | fc1t2 | fc2tb
    cbf = np.zeros((128, 1300), np.float32)
    for ep in range(2):
        cbf[:, ep * 640:ep * 640 + 512] = wk[:, ep * 512:(ep + 1) * 512]
        cbf[:, ep * 640 + 512:ep * 640 + 640] = wks[:, ep * 128:(ep + 1) * 128]
    fc1t = fc1_w.T / float(H * W)                          # [64, 16]
    cbf[:, 1280:1296] = np.vstack([fc1t, fc1t])            # K=128 sums rt halves
    cbf[0:16, 1296:1300] = fc2_w.T                         # [16, 4]
    cbf[16, 1296:1300] = fc2_b                             # bias row (K=17)

    # packed f32 consts: idc | bnb | fc2b
    cf32 = np.zeros((128, 131), np.float32)
    cf32[:, 0:128] = np.concatenate([
        (np.arange(128)[:, None] % 64 == np.arange(64)[None, :]),
        np.zeros((128, 64), bool)], axis=1).astype(np.float32)
    cf32[:, 128] = np.concatenate([shift[0], shift[1]])
    cf32[:, 129] = np.concatenate([shift[2], shift[3]])
    cf32[0:4, 130] = fc2_b

    return {"cbf": cbf.astype(BF16_NP), "cf32": cf32}


def kernel(x, fc1_w, fc2_w, fc2_b, conv_w, bn_gamma, bn_beta, bn_mean, bn_var):
    from concourse.bass_utils import run_bass_kernel_spmd

    x = np.asarray(x, np.float32)
    reps = int(os.environ.get("BASS_KERNEL_REPS", "1"))
    key = f"nc{reps}"
    if key not in _CACHE:
        _CACHE[key] = _build_program(reps)
    nc = _CACHE[key]

    wmap = _prep_weights(np.asarray(fc1_w, np.float32), np.asarray(fc2_w, np.float32),
                         np.asarray(fc2_b, np.float32), np.asarray(conv_w, np.float32),
                         np.asarray(bn_gamma, np.float32), np.asarray(bn_beta, np.float32),
                         np.asarray(bn_mean, np.float32), np.asarray(bn_var, np.float32))

    # host-side zero-pad + bf16 cast into the flat 162x162 (+margin) layout,
    # then 4 tap-shifted copies (shifts 0/1/2/164) so each SBUF region loads
    # as one full-width 128-partition DMA
    xp = np.zeros((B, 4, C, LHOST), BF16_NP)
    xpad = xp[:, 0, :, :LP].reshape(B, C, H + 2, WP)
    xpad[:, :, 1:H + 1, 1:W + 1] = x.astype(BF16_NP)
    for j, sft in ((1, 1), (2, 2), (3, 164)):
        xp[:, j, :, :LHOST - sft] = xp[:, 0, :, sft:]
    # fp8 copy for the router mean-pool (noise ~1e-4 of the logits)
    xq = np.zeros((B, C, LHOST), FP8_NP)
    xq[:, :, :LP] = xp[:, 0, :, :LP].astype(FP8_NP)

    in_maps = []
    for c in range(NCORES):
        m = dict(wmap)
        m["xp"] = xp[c * IMG_PER_CORE:(c + 1) * IMG_PER_CORE]
        m["xq"] = xq[c * IMG_PER_CORE:(c + 1) * IMG_PER_CORE]
        in_maps.append(m)

    trace = bool(int(os.environ.get("BASS_KERNEL_TRACE", "0")))
    res = run_bass_kernel_spmd(nc, in_maps, list(range(NCORES)), trace=trace)
    _CACHE["last_results"] = res
    return np.concatenate([res.results[c]["out"] for c in range(NCORES)], axis=0)


# revision 22
# speedup vs baseline: 1.0280x; 1.0280x over previous
"""Trainium2 Bass kernel for nn_DyConv (MoE routed dynamic conv).

Model (per batch image b):
  g = mean(x[b], spatial)                      # [C]
  w = softmax(fc2(relu(fc1(g))))               # [E]  router weights
  out[b] = sum_e w[e] * silu(bn_e(conv3x3_e(x[b])))

Strategy: pure data-parallel over batch. B=16 images / 8 cores = 2 images
per core; router + experts replicated. No collectives.

Per-core device program (per image):
  - x arrives host-padded to a flat 162x162 zero-padded layout in bf16.
  - Two SBUF "region" copies per half-image stack 2 shifted taps on the
    128 partitions: A = [x(+0); x(+1)], B = [x(+2); x(+164)].  A K=128
    matmul against A at offset o contracts taps (o, o+1) for all 64
    channels; 9 conv taps = 5 matmuls (3xA-pair, 1xB-pair, 1 half-K
    single) per expert-pair.  Two expert pairs stacked in M=128.
  - BN scale folded into conv weights on host; BN shift applied as the
    per-partition bias of the SiLU activation (ScalarE, PSUM->SBUF bf16).
  - Router: DVE chunk-reduces over the bf16 image; tiny matmuls + a
    tanh-based exp for the 4-way softmax; the mixing weights become two
    scaled-identity bf16 [128,64] lhsT tiles built on ScalarE.  The
    router is emitted in 4 stages one conv-tile apart, and image s+1's
    router is computed in the middle of image s.
  - Mix: 2 accumulating matmuls -> PSUM [64,N]; DVE compacting copy
    (drops the 2 pad cols) into a 3-tile (9-row) bounce group; one
    contiguous DMA per group to the output.
  - The TileScheduler reorders aggressively, so every order-sensitive
    instruction (DVE stream, DMA dispatch chains per queue, router
    stage ops) carries explicit sync=False ordering edges pinning it
    to its intended queue position.
"""
import os
import sys
import numpy as np

if "/opt/trn_rl_repo" not in sys.path:
    sys.path.insert(0, "/opt/trn_rl_repo")

import ml_dtypes  # noqa: E402

BF16_NP = ml_dtypes.bfloat16
FP8_NP = ml_dtypes.float8_e4m3fn

B, C, H, W = 16, 64, 160, 160
E, R = 4, 16
NCORES = 8
IMG_PER_CORE = B // NCORES          # 2
WP = W + 2                          # 162 padded row
LP = (H + 2) * WP                   # 26244 padded flat image
LHOST = 26600                       # host buffer with zero margin (max read 26408)
RLEN = 82 * WP                      # 13284: half-image region (80 out rows + 2 halo)
HB = 80 * WP                        # 12960: out-grid columns per half
NT = 486                            # psum tile = 3 out rows
BN_EPS = 1e-3

# chunk boundaries within a half-image region
CH5 = [0, 830, 1661, 3321, 6642, 9963, 13284]  # first-needed regions: small head
CH3 = [0, 830, 1661, 6642, 13284]
CH2 = [0, 6642, 13284]                      # steady-state prefetch loads
# router tile: half-image split across the partition dim so the DVE
# reduce covers the half in RTC columns; rows 0:64 and 64:128 are summed
# for free by the K=128 router matmul (fc1^T stacked twice).
RTC = 6642                                  # rt tile columns (h1 half size)
RTH = {0: 6480, 1: 6642}                    # per-half split point (h0: 80 rows)

_CACHE = {}


def _build_program(reps=1):
    import concourse.bacc as bacc
    import concourse.tile as tile
    from concourse import mybir
    from concourse.tile_rust import add_dep_helper

    BF16 = mybir.dt.bfloat16
    F32 = mybir.dt.float32
    AF = mybir.ActivationFunctionType
    ALU = mybir.AluOpType
    AX = mybir.AxisListType

    nc = bacc.Bacc("TRN2", target_bir_lowering=False, debug=False,
                   num_devices=NCORES)

    FP8 = mybir.dt.float8e4
    xp_d = nc.dram_tensor("xp", [IMG_PER_CORE, 4, C, LHOST], BF16, kind="ExternalInput")
    xq_d = nc.dram_tensor("xq", [IMG_PER_CORE, C, LHOST], FP8, kind="ExternalInput")
    cbf_d = nc.dram_tensor("cbf", [128, 1300], BF16, kind="ExternalInput")
    cf32_d = nc.dram_tensor("cf32", [128, 131], F32, kind="ExternalInput")
    out_d = nc.dram_tensor("out", [IMG_PER_CORE, C, H, W], F32, kind="ExternalOutput")

    seq = [i % IMG_PER_CORE for i in range(IMG_PER_CORE * reps)]
    S = len(seq)

    with tile.TileContext(nc) as tc:
        with tc.tile_pool(name="consts", bufs=1) as cp, \
             tc.tile_pool(name="regs", bufs=2) as rp, \
             tc.tile_pool(name="work", bufs=2) as wp, \
             tc.tile_pool(name="bounce", bufs=3) as bp, \
             tc.tile_pool(name="psum", bufs=1, space="PSUM") as pp:

            cbf = cp.tile([128, 1300], BF16)
            cb1 = nc.sync.dma_start(cbf[:, 0:640], cbf_d[:, 0:640])
            cb2 = nc.sync.dma_start(cbf[:, 640:1300], cbf_d[:, 640:1300])
            add_dep_helper(cb2.ins, cb1.ins, sync=False, reason="order-pin")
            cf32 = cp.tile([128, 131], F32)
            last_cf = nc.gpsimd.dma_start(cf32[:, :], cf32_d[:, :])
            o64_sb = cp.tile([1, 64], F32)
            nc.gpsimd.memset(o64_sb[:, :], 1.0)

            fc1t_sb = cbf[0:128, 1280:1296]
            fc2t_sb = cbf[0:17, 1296:1300]
            idc_sb = cf32[:, 0:128]
            bnb_sb = cf32[:, 128:130]
            fc2b_sb = cf32[0:4, 130:131]

            regA_t = {}
            regB_t = {}
            parts_t = {}
            mixw_of = {}
            routst = {}
            # ordering-pin state: last instruction per in-order stream
            last = {"dve": None, "gp": last_cf, "sc": None, "syb": None,
                    "out": None, "conv": None, "silu": None}

            def dep(a, b):
                if a is not None and b is not None:
                    add_dep_helper(a.ins, b.ins, sync=False, reason="order-pin")

            def dve(inst):
                dep(inst, last["dve"])
                last["dve"] = inst
                return inst

            def disp_region(s, h, reg, qkey, chb, extra_anchor=None):
                img = seq[s]
                if reg == "A":
                    if (s, h) not in regA_t:
                        regA_t[(s, h)] = rp.tile([128, RLEN], BF16, tag="regA",
                                                 name=f"regA{s}h{h}", bufs=3)
                    t = regA_t[(s, h)]
                    j0 = 0
                else:
                    if (s, h) not in regB_t:
                        regB_t[(s, h)] = rp.tile([128, RLEN], BF16, tag="regB",
                                                 name=f"regB{s}h{h}", bufs=2)
                    t = regB_t[(s, h)]
                    j0 = 2
                eng = {"gp": nc.gpsimd, "sc": nc.scalar, "syb": nc.sync}[qkey]
                for c in range(len(chb) - 1):
                    i = eng.dma_start(
                        t[:, chb[c]:chb[c + 1]],
                        xp_d[img, j0:j0 + 2, :,
                             h * HB + chb[c]:h * HB + chb[c + 1]]
                        .rearrange("j c f -> (j c) f"))
                    dep(i, last[qkey])
                    dep(i, extra_anchor)
                    last[qkey] = i

            rt_t = {}

            def disp_rt(s, h, qkey, extra_anchor=None):
                # fp8 router tile: rows 0:64 = first RTH[h] cols of the
                # half, rows 64:128 = the rest; fc1^T rows stacked 2x so
                # the K=128 matmul sums both halves for free
                img = seq[s]
                t = rp.tile([128, RTC], FP8, tag="rt", name=f"rt{s}h{h}",
                            bufs=2)
                rt_t[(s, h)] = t
                L = RTH[h]
                eng = {"gp": nc.gpsimd, "sc": nc.scalar, "syb": nc.sync}[qkey]
                for r0, off in ((0, 0), (64, L)):
                    i = eng.dma_start(
                        t[r0:r0 + 64, 0:L],
                        xq_d[img, :, h * HB + off:h * HB + off + L])
                    dep(i, last[qkey])
                    dep(i, extra_anchor)
                    last[qkey] = i

            def emit_reduce_chunk(s, h, c):
                # partial router sum over one rt column chunk (DVE, 128 rows)
                if s not in parts_t:
                    parts_t[s] = wp.tile([128, 4], F32, tag="parts", bufs=2,
                                         name=f"parts{s}")
                L = RTH[h]
                a, b = c * L // 2, (c + 1) * L // 2
                dve(nc.vector.tensor_reduce(
                    parts_t[s][:, h * 2 + c:h * 2 + c + 1],
                    rt_t[(s, h)][:, a:b],
                    axis=AX.X, op=ALU.add))

            def emit_router_stage(s, k):
                st = routst.setdefault(s, {})
                pconv, psilu = last["conv"], last["silu"]
                if k == 0:
                    st["gsum"] = wp.tile([128, 1], F32, tag="gsum", name=f"gsum{s}")
                    dve(nc.vector.tensor_reduce(st["gsum"][:, :], parts_t[s][:, 0:4],
                                                axis=AX.X, op=ALU.add))
                    st["gbf"] = wp.tile([128, 1], BF16, tag="gbf", name=f"gbf{s}")
                    dve(nc.vector.tensor_copy(st["gbf"][:, :], st["gsum"][:, :]))
                    h_ps = pp.tile([16, 1], F32, tag="pr", name=f"hps{s}")
                    dep(nc.tensor.matmul(h_ps[:, :], fc1t_sb, st["gbf"][:, :],
                                         start=True, stop=True), pconv)
                    st["hbf"] = wp.tile([32, 1], BF16, tag="hbf", name=f"hbf{s}")
                    dve(nc.vector.memset(st["hbf"][:, :], 1.0))
                    dep(nc.scalar.activation(st["hbf"][0:16, :], h_ps[:, :], AF.Relu),
                        psilu)
                elif k == 1:
                    # logits as a row: [hbf;1]^T @ [fc2^T; fc2b^T]  (K=17)
                    lrow_ps = pp.tile([1, 4], F32, tag="pr", name=f"lrow{s}")
                    dep(nc.tensor.matmul(lrow_ps[:, :], st["hbf"][0:17, :],
                                         fc2t_sb, start=True, stop=True), pconv)
                    st["lrow"] = lrow_ps
                elif k == 2:
                    # exp(l) = (1 + tanh(l/2)) / (1 - tanh(l/2)); logits are O(0.5)
                    trow = wp.tile([1, 4], F32, tag="trow", name=f"trow{s}")
                    dep(nc.scalar.activation(trow[:, :], st["lrow"][:, :], AF.Tanh,
                                             scale=0.5), psilu)
                    num = wp.tile([1, 4], F32, tag="num", name=f"num{s}")
                    dve(nc.vector.tensor_scalar_add(num[:, :], trow[:, :], 1.0))
                    den = wp.tile([1, 4], F32, tag="den", name=f"den{s}")
                    dve(nc.vector.tensor_scalar(den[:, :], trow[:, :], -1.0, 1.0,
                                                op0=ALU.mult, op1=ALU.add))
                    rec = wp.tile([1, 4], F32, tag="rec", name=f"rec{s}")
                    dve(nc.vector.reciprocal(rec[:, :], den[:, :]))
                    erow = wp.tile([1, 4], F32, tag="erow", name=f"erow{s}")
                    dve(nc.vector.tensor_tensor(erow[:, :], num[:, :], rec[:, :],
                                                op=ALU.mult))
                    ssum = wp.tile([1, 1], F32, tag="ssum", name=f"ssum{s}")
                    dve(nc.vector.tensor_reduce(ssum[:, :], erow[:, :], axis=AX.X,
                                                op=ALU.add))
                    sinv = wp.tile([1, 1], F32, tag="sinv", name=f"sinv{s}")
                    dve(nc.vector.reciprocal(sinv[:, :], ssum[:, :]))
                    wrow = wp.tile([1, 4], F32, tag="wrow", name=f"wrow{s}")
                    dve(nc.vector.tensor_scalar_mul(wrow[:, :], erow[:, :],
                                                    sinv[:, 0:1]))
                    rowA = wp.tile([1, 128], F32, tag="rowA", name=f"rowA{s}")
                    dve(nc.vector.tensor_scalar_mul(rowA[:, 0:64], o64_sb[:, :],
                                                    wrow[:, 0:1]))
                    dve(nc.vector.tensor_scalar_mul(rowA[:, 64:128], o64_sb[:, :],
                                                    wrow[:, 1:2]))
                    rowB = wp.tile([1, 128], F32, tag="rowB", name=f"rowB{s}")
                    dve(nc.vector.tensor_scalar_mul(rowB[:, 0:64], o64_sb[:, :],
                                                    wrow[:, 2:3]))
                    dve(nc.vector.tensor_scalar_mul(rowB[:, 64:128], o64_sb[:, :],
                                                    wrow[:, 3:4]))
                    st["rowA"], st["rowB"] = rowA, rowB
                else:
                    wc_ps = pp.tile([128, 2], F32, tag="pr", name=f"wc{s}")
                    dep(nc.tensor.matmul(wc_ps[:, 0:1], st["rowA"][:, :],
                                         idc_sb[0:1, 0:1], start=True, stop=True),
                        pconv)
                    dep(nc.tensor.matmul(wc_ps[:, 1:2], st["rowB"][:, :],
                                         idc_sb[0:1, 0:1], start=True, stop=True),
                        pconv)
                    wcol = wp.tile([128, 2], F32, tag="wcol", name=f"wcol{s}")
                    dep(nc.scalar.copy(wcol[:, :], wc_ps[:, :]), psilu)
                    mA = wp.tile([128, 128], BF16, tag="mixA", name=f"mixA{s}")
                    dep(nc.scalar.activation(mA[:, :], idc_sb, AF.Copy,
                                             scale=wcol[:, 0:1]), psilu)
                    mB = wp.tile([128, 128], BF16, tag="mixB", name=f"mixB{s}")
                    dep(nc.scalar.activation(mB[:, :], idc_sb, AF.Copy,
                                             scale=wcol[:, 1:2]), psilu)
                    mixw_of[s] = (mA, mB)

            # ---- mix + compacting copy + grouped output DMA ----
            grp_state = {}

            def emit_mix(st):
                tsbs, N, h, t, s = st
                img = seq[s]
                mA, mB = mixw_of[s]
                po = pp.tile([128, N], F32, tag="po", bufs=3)
                nc.tensor.matmul(po[:, :], mA[:, :], tsbs[0][:, :],
                                 start=True, stop=False)
                nc.tensor.matmul(po[:, :], mB[:, :], tsbs[1][:, :],
                                 start=False, stop=True)
                g, j = divmod(t, 3)
                if j == 0:
                    grp_state["t"] = bp.tile([64, 1440], F32, tag="bounce",
                                             name="bounce", bufs=3)
                bt = grp_state["t"]
                nrows = 3 if t < 26 else 2
                src = po[0:64, :].rearrange("p (r c) -> p r c", c=WP)[:, 0:nrows, 0:W]
                dst = bt[:, j * 480:j * 480 + nrows * W].rearrange(
                    "p (r c) -> p r c", c=W)
                dve(nc.vector.tensor_copy(dst, src))
                if s == S - 1 and h == 1 and t >= 24:
                    r0 = h * 80 + t * 3
                    i = nc.sync.dma_start(
                        out_d[img, :, r0:r0 + nrows, :],
                        bt[:, j * 480:j * 480 + nrows * W].rearrange(
                            "p (r c) -> p r c", c=W))
                    dep(i, last["out"])
                    last["out"] = i
                elif j == 2 or t == 26:
                    r0 = h * 80 + g * 9
                    nr = 9 if g < 8 else 8
                    i = nc.sync.dma_start(
                        out_d[img, :, r0:r0 + nr, :],
                        bt[:, 0:nr * W].rearrange("p (r c) -> p r c", c=W))
                    dep(i, last["out"])
                    last["out"] = i

            # ---- main loop over image sequence ----
            pending = []
            for s in range(S):
                if s == 0:
                    # <=8 dispatches per shared sem pool at startup:
                    # gpsimd: cf32,A00x5,A01x2; scalar: B00x3; sync:
                    # cbf,rt00x2,rt01x2.  B01 follows at gtile 6.
                    disp_region(0, 0, "A", "gp", CH5[0:5])
                    disp_region(0, 1, "A", "gp", CH2)
                    disp_region(0, 0, "B", "sc", CH3)
                else:
                    # h1's B region: its buffer frees at image s-1's end
                    disp_region(s, 1, "B", "syb", CH2, extra_anchor=last["out"])
                if s + 1 < S:
                    disp_region(s + 1, 0, "A", "gp", CH2)
                ROUTER_G = 9 if s == 0 else -10  # stages 9-12, mixes from 13
                for h in (0, 1):
                    regA = regA_t[(s, h)]
                    regB = regB_t[(s, h)]
                    for t in range(27):
                        gtile = h * 27 + t
                        if s == 0:
                            if gtile == 2:
                                disp_rt(0, 0, "syb")
                            elif gtile == 3:
                                emit_reduce_chunk(0, 0, 0)
                                emit_reduce_chunk(0, 0, 1)
                            elif gtile == 4:
                                disp_rt(0, 1, "syb")
                            elif gtile == 5:
                                emit_reduce_chunk(0, 1, 0)
                                emit_reduce_chunk(0, 1, 1)
                                disp_region(0, 0, "A", "syb", CH5[4:6])
                            elif gtile == 8:
                                disp_region(0, 0, "A", "syb", CH5[5:7])
                            elif gtile == 6:
                                disp_region(0, 1, "B", "sc", CH2,
                                            extra_anchor=last["silu"])
                        if s + 1 < S:
                            if gtile == 26:
                                disp_rt(s + 1, 0, "sc",
                                        extra_anchor=last["silu"])
                            elif gtile == 27:
                                disp_rt(s + 1, 1, "sc",
                                        extra_anchor=last["silu"])
                            elif gtile == 28:
                                disp_region(s + 1, 0, "B", "syb", CH2,
                                            extra_anchor=last["out"])
                            elif gtile == 30:
                                disp_region(s + 1, 1, "A", "sc", CH2,
                                            extra_anchor=last["silu"])
                            elif gtile in (38, 39):
                                emit_reduce_chunk(s + 1, 0, gtile - 38)
                            elif gtile in (44, 45):
                                emit_reduce_chunk(s + 1, 1, gtile - 44)
                            elif 48 <= gtile <= 51:
                                emit_router_stage(s + 1, gtile - 48)
                        N = NT if t < 26 else 324
                        n0 = t * NT
                        tsbs = []
                        for ep in range(2):
                            cps = pp.tile([128, N], F32, tag=f"pc{ep}", bufs=2)
                            base = ep * 640
                            nc.tensor.matmul(cps[:, :], cbf[:, base:base + 128],
                                             regA[:, n0:n0 + N], start=True, stop=False)
                            nc.tensor.matmul(cps[:, :], cbf[:, base + 128:base + 256],
                                             regA[:, n0 + 162:n0 + 162 + N],
                                             start=False, stop=False)
                            nc.tensor.matmul(cps[:, :], cbf[:, base + 256:base + 384],
                                             regA[:, n0 + 324:n0 + 324 + N],
                                             start=False, stop=False)
                            nc.tensor.matmul(cps[:, :], cbf[:, base + 384:base + 512],
                                             regB[:, n0:n0 + N], start=False, stop=False)
                            mlast = nc.tensor.matmul(
                                cps[:, :], cbf[:, base + 512:base + 640],
                                regB[:, n0 + 324:n0 + 324 + N],
                                start=False, stop=True)
                            tsb = wp.tile([128, N], BF16, tag=f"t{ep}", bufs=15)
                            slast = nc.scalar.activation(tsb[:, :], cps[:, :], AF.Silu,
                                                         bias=bnb_sb[:, ep:ep + 1])
                            tsbs.append(tsb)
                        last["conv"], last["silu"] = mlast, slast
                        pending.append((tsbs, N, h, t, s))
                        if 0 <= gtile - ROUTER_G <= 3:
                            emit_router_stage(s, gtile - ROUTER_G)
                        if s > 0 or gtile > ROUTER_G + 3:
                            k = 0
                            while len(pending) > 2 and k < 2:
                                emit_mix(pending.pop(0))
                                k += 1
                # drain this image's tail so bounce groups stay h-aligned
                while pending:
                    emit_mix(pending.pop(0))

    nc.compile()
    return nc


def _prep_weights(fc1_w, fc2_w, fc2_b, conv_w, bn_gamma, bn_beta, bn_mean, bn_var):
    scale = bn_gamma / np.sqrt(bn_var + BN_EPS)            # [E, C]
    shift = bn_beta - bn_mean * scale                      # [E, C]
    ws = conv_w * scale[:, :, None, None, None]            # [E, Co, Ci, 3, 3]

    # paired-tap lhsT blocks: [K=128 (2 taps x 64 ci), M=128 (2 experts x 64 co)]
    groups = [((0, 0), (0, 1)), ((1, 0), (1, 1)), ((2, 0), (2, 1)), ((0, 2), (1, 2))]
    wk = np.zeros((128, 1024), np.float32)
    for ep in range(2):
        for g, (ta, tb) in enumerate(groups):
            blk = np.stack([ws[:, :, :, ta[0], ta[1]], ws[:, :, :, tb[0], tb[1]]])
            blk = blk[:, 2 * ep:2 * ep + 2]                # [j, le, Co, Ci]
            lhsT = blk.transpose(0, 3, 1, 2).reshape(128, 128)
            wk[:, (ep * 4 + g) * 128:(ep * 4 + g + 1) * 128] = lhsT
    wks = np.zeros((128, 256), np.float32)                 # rows 64-127 stay zero
    s22 = ws[:, :, :, 2, 2]                                # [E, Co, Ci]
    for ep in range(2):
        blk = s22[2 * ep:2 * ep + 2]                       # [le, Co, Ci]
        wks[0:64, ep * 128:(ep + 1) * 128] = blk.transpose(2, 0, 1).reshape(64, 128)

    # packed bf16 consts: per-ep [wk 512 | wks 128] | fc1t2 | fc2tb
    cbf = np.zeros((128, 1300), np.float32)
    for ep in range(2):
        cbf[:, ep * 640:ep * 640 + 512] = wk[:, ep * 512:(ep + 1) * 512]
        cbf[:, ep * 640 + 512:ep * 640 + 640] = wks[:, ep * 128:(ep + 1) * 128]
    fc1t = fc1_w.T / float(H * W)                          # [64, 16]
    cbf[:, 1280:1296] = np.vstack([fc1t, fc1t])            # K=128 sums rt halves
    cbf[0:16, 1296:1300] = fc2_w.T                         # [16, 4]
    cbf[16, 1296:1300] = fc2_b                             # bias row (K=17)

    # packed f32 consts: idc | bnb | fc2b
    cf32 = np.zeros((128, 131), np.float32)
    cf32[:, 0:128] = np.concatenate([
        (np.arange(128)[:, None] % 64 == np.arange(64)[None, :]),
        np.zeros((128, 64), bool)], axis=1).astype(np.float32)
    cf32[:, 128] = np.concatenate([shift[0], shift[1]])
    cf32[:, 129] = np.concatenate([shift[2], shift[3]])
    cf32[0:4, 130] = fc2_b

    return {"cbf": cbf.astype(BF16_NP), "cf32": cf32}


def kernel(x, fc1_w, fc2_w, fc2_b, conv_w, bn_gamma, bn_beta, bn_mean, bn_var):
    from concourse.bass_utils import run_bass_kernel_spmd

    x = np.asarray(x, np.float32)
    reps = int(os.environ.get("BASS_KERNEL_REPS", "1"))
    key = f"nc{reps}"
    if key not in _CACHE:
        _CACHE[key] = _build_program(reps)
    nc = _CACHE[key]

    wmap = _prep_weights(np.asarray(fc1_w, np.float32), np.asarray(fc2_w, np.float32),
                         np.asarray(fc2_b, np.float32), np.asarray(conv_w, np.float32),
                         np.asarray(bn_gamma, np.float32), np.asarray(bn_beta, np.float32),
                         np.asarray(bn_mean, np.float32), np.asarray(bn_var, np.float32))

    # host-side zero-pad + bf16 cast into the flat 162x162 (+margin) layout,
    # then 4 tap-shifted copies (shifts 0/1/2/164) so each SBUF region loads
    # as one full-width 128-partition DMA
    xp = np.zeros((B, 4, C, LHOST), BF16_NP)
    xpad = xp[:, 0, :, :LP].reshape(B, C, H + 2, WP)
    xpad[:, :, 1:H + 1, 1:W + 1] = x.astype(BF16_NP)
    for j, sft in ((1, 1), (2, 2), (3, 164)):
        xp[:, j, :, :LHOST - sft] = xp[:, 0, :, sft:]
    # fp8 copy for the router mean-pool (noise ~1e-4 of the logits)
    xq = np.zeros((B, C, LHOST), FP8_NP)
    xq[:, :, :LP] = xp[:, 0, :, :LP].astype(FP8_NP)

    in_maps = []
    for c in range(NCORES):
        m = dict(wmap)
        m["xp"] = xp[c * IMG_PER_CORE:(c + 1) * IMG_PER_CORE]
        m["xq"] = xq[c * IMG_PER_CORE:(c + 1) * IMG_PER_CORE]
        in_maps.append(m)

    trace = bool(int(os.environ.get("BASS_KERNEL_TRACE", "0")))
    res = run_bass_kernel_spmd(nc, in_maps, list(range(NCORES)), trace=trace)
    _CACHE["last_results"] = res
    return np.concatenate([res.results[c]["out"] for c in range(NCORES)], axis=0)


# revision 23
# speedup vs baseline: 1.0457x; 1.0173x over previous
"""Trainium2 Bass kernel for nn_DyConv (MoE routed dynamic conv).

Model (per batch image b):
  g = mean(x[b], spatial)                      # [C]
  w = softmax(fc2(relu(fc1(g))))               # [E]  router weights
  out[b] = sum_e w[e] * silu(bn_e(conv3x3_e(x[b])))

Strategy: pure data-parallel over batch. B=16 images / 8 cores = 2 images
per core; router + experts replicated. No collectives.

Per-core device program (per image):
  - x arrives host-padded to a flat 162x162 zero-padded layout in bf16.
  - Two SBUF "region" copies per half-image stack 2 shifted taps on the
    128 partitions: A = [x(+0); x(+1)], B = [x(+2); x(+164)].  A K=128
    matmul against A at offset o contracts taps (o, o+1) for all 64
    channels; 9 conv taps = 5 matmuls (3xA-pair, 1xB-pair, 1 half-K
    single) per expert-pair.  Two expert pairs stacked in M=128.
  - BN scale folded into conv weights on host; BN shift applied as the
    per-partition bias of the SiLU activation (ScalarE, PSUM->SBUF bf16).
  - Router: DVE chunk-reduces over the bf16 image; tiny matmuls + a
    tanh-based exp for the 4-way softmax; the mixing weights become two
    scaled-identity bf16 [128,64] lhsT tiles built on ScalarE.  The
    router is emitted in 4 stages one conv-tile apart, and image s+1's
    router is computed in the middle of image s.
  - Mix: 2 accumulating matmuls -> PSUM [64,N]; DVE compacting copy
    (drops the 2 pad cols) into a 3-tile (9-row) bounce group; one
    contiguous DMA per group to the output.
  - The TileScheduler reorders aggressively, so every order-sensitive
    instruction (DVE stream, DMA dispatch chains per queue, router
    stage ops) carries explicit sync=False ordering edges pinning it
    to its intended queue position.
"""
import os
import sys
import numpy as np

if "/opt/trn_rl_repo" not in sys.path:
    sys.path.insert(0, "/opt/trn_rl_repo")

import ml_dtypes  # noqa: E402

BF16_NP = ml_dtypes.bfloat16
FP8_NP = ml_dtypes.float8_e4m3fn

B, C, H, W = 16, 64, 160, 160
E, R = 4, 16
NCORES = 8
IMG_PER_CORE = B // NCORES          # 2
WP = W + 2                          # 162 padded row
LP = (H + 2) * WP                   # 26244 padded flat image
LHOST = 26600                       # host buffer with zero margin (max read 26408)
RLEN = 82 * WP                      # 13284: half-image region (80 out rows + 2 halo)
HB = 80 * WP                        # 12960: out-grid columns per half
NT = 486                            # psum tile = 3 out rows
BN_EPS = 1e-3

# chunk boundaries within a half-image region
CH5 = [0, 830, 1661, 3321, 6642, 9963, 13284]  # first-needed regions: small head
CH3 = [0, 830, 1661, 6642, 13284]
CH2 = [0, 6642, 13284]                      # steady-state prefetch loads
# router tile: half-image split across the partition dim so the DVE
# reduce covers the half in RTC columns; rows 0:64 and 64:128 are summed
# for free by the K=128 router matmul (fc1^T stacked twice).
RTC = 6642                                  # rt tile columns (h1 half size)
RTH = {0: 6480, 1: 6642}                    # per-half split point (h0: 80 rows)

_CACHE = {}


def _build_program(reps=1):
    import concourse.bacc as bacc
    import concourse.tile as tile
    from concourse import mybir
    from concourse.tile_rust import add_dep_helper

    BF16 = mybir.dt.bfloat16
    F32 = mybir.dt.float32
    AF = mybir.ActivationFunctionType
    ALU = mybir.AluOpType
    AX = mybir.AxisListType

    nc = bacc.Bacc("TRN2", target_bir_lowering=False, debug=False,
                   num_devices=NCORES)

    FP8 = mybir.dt.float8e4
    xp_d = nc.dram_tensor("xp", [IMG_PER_CORE, 4, C, LHOST], BF16, kind="ExternalInput")
    xq_d = nc.dram_tensor("xq", [IMG_PER_CORE, C, LHOST], FP8, kind="ExternalInput")
    cbf_d = nc.dram_tensor("cbf", [128, 1300], BF16, kind="ExternalInput")
    cf32_d = nc.dram_tensor("cf32", [128, 131], F32, kind="ExternalInput")
    out_d = nc.dram_tensor("out", [IMG_PER_CORE, C, H, W], F32, kind="ExternalOutput")

    seq = [i % IMG_PER_CORE for i in range(IMG_PER_CORE * reps)]
    S = len(seq)

    with tile.TileContext(nc) as tc:
        with tc.tile_pool(name="consts", bufs=1) as cp, \
             tc.tile_pool(name="regs", bufs=2) as rp, \
             tc.tile_pool(name="work", bufs=2) as wp, \
             tc.tile_pool(name="bounce", bufs=3) as bp, \
             tc.tile_pool(name="psum", bufs=1, space="PSUM") as pp:

            cbf = cp.tile([128, 1300], BF16)
            cb1 = nc.sync.dma_start(cbf[:, 0:640], cbf_d[:, 0:640])
            cb2 = nc.sync.dma_start(cbf[:, 640:1300], cbf_d[:, 640:1300])
            add_dep_helper(cb2.ins, cb1.ins, sync=False, reason="order-pin")
            cf32 = cp.tile([128, 131], F32)
            last_cf = nc.gpsimd.dma_start(cf32[:, :], cf32_d[:, :])
            o64_sb = cp.tile([1, 64], F32)
            nc.gpsimd.memset(o64_sb[:, :], 1.0)
            ob1_sb = cp.tile([1, 1], BF16)
            nc.gpsimd.memset(ob1_sb[:, :], 1.0)

            fc1t_sb = cbf[0:128, 1280:1296]
            fc2t_sb = cbf[0:17, 1296:1300]
            idc_sb = cf32[:, 0:128]
            bnb_sb = cf32[:, 128:130]
            fc2b_sb = cf32[0:4, 130:131]

            regA_t = {}
            regB_t = {}
            parts_t = {}
            mixw_of = {}
            routst = {}
            # ordering-pin state: last instruction per in-order stream
            last = {"dve": None, "gp": last_cf, "sc": None, "syb": None,
                    "out": None, "conv": None, "silu": None}

            def dep(a, b):
                if a is not None and b is not None:
                    add_dep_helper(a.ins, b.ins, sync=False, reason="order-pin")

            def dve(inst):
                dep(inst, last["dve"])
                last["dve"] = inst
                return inst

            def disp_region(s, h, reg, qkey, chb, extra_anchor=None):
                img = seq[s]
                if reg == "A":
                    if (s, h) not in regA_t:
                        regA_t[(s, h)] = rp.tile([128, RLEN], BF16, tag="regA",
                                                 name=f"regA{s}h{h}", bufs=3)
                    t = regA_t[(s, h)]
                    j0 = 0
                else:
                    if (s, h) not in regB_t:
                        regB_t[(s, h)] = rp.tile([128, RLEN], BF16, tag="regB",
                                                 name=f"regB{s}h{h}", bufs=2)
                    t = regB_t[(s, h)]
                    j0 = 2
                eng = {"gp": nc.gpsimd, "sc": nc.scalar, "syb": nc.sync}[qkey]
                for c in range(len(chb) - 1):
                    i = eng.dma_start(
                        t[:, chb[c]:chb[c + 1]],
                        xp_d[img, j0:j0 + 2, :,
                             h * HB + chb[c]:h * HB + chb[c + 1]]
                        .rearrange("j c f -> (j c) f"))
                    dep(i, last[qkey])
                    dep(i, extra_anchor)
                    last[qkey] = i

            rt_t = {}

            def disp_rt(s, h, qkey, extra_anchor=None):
                # fp8 router tile: rows 0:64 = first RTH[h] cols of the
                # half, rows 64:128 = the rest; fc1^T rows stacked 2x so
                # the K=128 matmul sums both halves for free
                img = seq[s]
                t = rp.tile([128, RTC], FP8, tag="rt", name=f"rt{s}h{h}",
                            bufs=2)
                rt_t[(s, h)] = t
                L = RTH[h]
                eng = {"gp": nc.gpsimd, "sc": nc.scalar, "syb": nc.sync}[qkey]
                for r0, off in ((0, 0), (64, L)):
                    i = eng.dma_start(
                        t[r0:r0 + 64, 0:L],
                        xq_d[img, :, h * HB + off:h * HB + off + L])
                    dep(i, last[qkey])
                    dep(i, extra_anchor)
                    last[qkey] = i

            def emit_reduce_chunk(s, h, c):
                # partial router sum over one rt column chunk (DVE, 128 rows)
                if s not in parts_t:
                    parts_t[s] = wp.tile([128, 4], F32, tag="parts", bufs=2,
                                         name=f"parts{s}")
                L = RTH[h]
                a, b = c * L // 2, (c + 1) * L // 2
                dve(nc.vector.tensor_reduce(
                    parts_t[s][:, h * 2 + c:h * 2 + c + 1],
                    rt_t[(s, h)][:, a:b],
                    axis=AX.X, op=ALU.add))

            def emit_router_stage(s, k):
                st = routst.setdefault(s, {})
                pconv, psilu = last["conv"], last["silu"]
                if k == 0:
                    st["gsum"] = wp.tile([128, 1], F32, tag="gsum", name=f"gsum{s}")
                    dve(nc.vector.tensor_reduce(st["gsum"][:, :], parts_t[s][:, 0:4],
                                                axis=AX.X, op=ALU.add))
                    st["gbf"] = wp.tile([128, 1], BF16, tag="gbf", name=f"gbf{s}")
                    dve(nc.vector.tensor_copy(st["gbf"][:, :], st["gsum"][:, :]))
                    h_ps = pp.tile([16, 1], F32, tag="pr", name=f"hps{s}")
                    dep(nc.tensor.matmul(h_ps[:, :], fc1t_sb, st["gbf"][:, :],
                                         start=True, stop=True), pconv)
                    st["hbf"] = wp.tile([32, 1], BF16, tag="hbf", name=f"hbf{s}")
                    dve(nc.vector.memset(st["hbf"][:, :], 1.0))
                    dep(nc.scalar.activation(st["hbf"][0:16, :], h_ps[:, :], AF.Relu),
                        psilu)
                elif k == 1:
                    # logits as a row: [hbf;1]^T @ [fc2^T; fc2b^T]  (K=17)
                    lrow_ps = pp.tile([1, 4], F32, tag="pr", name=f"lrow{s}")
                    dep(nc.tensor.matmul(lrow_ps[:, :], st["hbf"][0:17, :],
                                         fc2t_sb, start=True, stop=True), pconv)
                    st["lrow"] = lrow_ps
                elif k == 2:
                    # exp(l) = (1 + tanh(l/2)) / (1 - tanh(l/2)); logits are O(0.5)
                    trow = wp.tile([1, 4], F32, tag="trow", name=f"trow{s}")
                    dep(nc.scalar.activation(trow[:, :], st["lrow"][:, :], AF.Tanh,
                                             scale=0.5), psilu)
                    num = wp.tile([1, 4], F32, tag="num", name=f"num{s}")
                    dve(nc.vector.tensor_scalar_add(num[:, :], trow[:, :], 1.0))
                    den = wp.tile([1, 4], F32, tag="den", name=f"den{s}")
                    dve(nc.vector.tensor_scalar(den[:, :], trow[:, :], -1.0, 1.0,
                                                op0=ALU.mult, op1=ALU.add))
                    rec = wp.tile([1, 4], F32, tag="rec", name=f"rec{s}")
                    dve(nc.vector.reciprocal(rec[:, :], den[:, :]))
                    erow = wp.tile([1, 4], F32, tag="erow", name=f"erow{s}")
                    dve(nc.vector.tensor_tensor(erow[:, :], num[:, :], rec[:, :],
                                                op=ALU.mult))
                    ssum = wp.tile([1, 1], F32, tag="ssum", name=f"ssum{s}")
                    dve(nc.vector.tensor_reduce(ssum[:, :], erow[:, :], axis=AX.X,
                                                op=ALU.add))
                    sinv = wp.tile([1, 1], F32, tag="sinv", name=f"sinv{s}")
                    dve(nc.vector.reciprocal(sinv[:, :], ssum[:, :]))
                    wrow = wp.tile([1, 4], F32, tag="wrow", name=f"wrow{s}")
                    dve(nc.vector.tensor_scalar_mul(wrow[:, :], erow[:, :],
                                                    sinv[:, 0:1]))
                    rowA = wp.tile([1, 128], BF16, tag="rowA", name=f"rowA{s}")
                    dve(nc.vector.tensor_scalar_mul(rowA[:, 0:64], o64_sb[:, :],
                                                    wrow[:, 0:1]))
                    dve(nc.vector.tensor_scalar_mul(rowA[:, 64:128], o64_sb[:, :],
                                                    wrow[:, 1:2]))
                    rowB = wp.tile([1, 128], BF16, tag="rowB", name=f"rowB{s}")
                    dve(nc.vector.tensor_scalar_mul(rowB[:, 0:64], o64_sb[:, :],
                                                    wrow[:, 2:3]))
                    dve(nc.vector.tensor_scalar_mul(rowB[:, 64:128], o64_sb[:, :],
                                                    wrow[:, 3:4]))
                    st["rowA"], st["rowB"] = rowA, rowB
                else:
                    wc_ps = pp.tile([128, 2], F32, tag="pr", name=f"wc{s}")
                    dep(nc.tensor.matmul(wc_ps[:, 0:1], st["rowA"][:, :],
                                         ob1_sb[:, :], start=True, stop=True),
                        pconv)
                    dep(nc.tensor.matmul(wc_ps[:, 1:2], st["rowB"][:, :],
                                         ob1_sb[:, :], start=True, stop=True),
                        pconv)
                    wcol = wp.tile([128, 2], F32, tag="wcol", name=f"wcol{s}")
                    dep(nc.scalar.copy(wcol[:, :], wc_ps[:, :]), psilu)
                    mA = wp.tile([128, 128], BF16, tag="mixA", name=f"mixA{s}")
                    dep(nc.scalar.activation(mA[:, :], idc_sb, AF.Copy,
                                             scale=wcol[:, 0:1]), psilu)
                    mB = wp.tile([128, 128], BF16, tag="mixB", name=f"mixB{s}")
                    dep(nc.scalar.activation(mB[:, :], idc_sb, AF.Copy,
                                             scale=wcol[:, 1:2]), psilu)
                    mixw_of[s] = (mA, mB)

            # ---- mix + compacting copy + grouped output DMA ----
            grp_state = {}

            def emit_mix(st):
                tsbs, N, h, t, s = st
                img = seq[s]
                mA, mB = mixw_of[s]
                po = pp.tile([128, N], F32, tag="po", bufs=3)
                nc.tensor.matmul(po[:, :], mA[:, :], tsbs[0][:, :],
                                 start=True, stop=False)
                nc.tensor.matmul(po[:, :], mB[:, :], tsbs[1][:, :],
                                 start=False, stop=True)
                g, j = divmod(t, 3)
                if j == 0:
                    grp_state["t"] = bp.tile([64, 1440], F32, tag="bounce",
                                             name="bounce", bufs=3)
                bt = grp_state["t"]
                nrows = 3 if t < 26 else 2
                src = po[0:64, :].rearrange("p (r c) -> p r c", c=WP)[:, 0:nrows, 0:W]
                dst = bt[:, j * 480:j * 480 + nrows * W].rearrange(
                    "p (r c) -> p r c", c=W)
                dve(nc.vector.tensor_copy(dst, src))
                if s == S - 1 and h == 1 and t >= 24:
                    r0 = h * 80 + t * 3
                    i = nc.sync.dma_start(
                        out_d[img, :, r0:r0 + nrows, :],
                        bt[:, j * 480:j * 480 + nrows * W].rearrange(
                            "p (r c) -> p r c", c=W))
                    dep(i, last["out"])
                    last["out"] = i
                elif j == 2 or t == 26:
                    r0 = h * 80 + g * 9
                    nr = 9 if g < 8 else 8
                    i = nc.sync.dma_start(
                        out_d[img, :, r0:r0 + nr, :],
                        bt[:, 0:nr * W].rearrange("p (r c) -> p r c", c=W))
                    dep(i, last["out"])
                    last["out"] = i

            # ---- main loop over image sequence ----
            pending = []
            for s in range(S):
                if s == 0:
                    # <=8 dispatches per shared sem pool at startup:
                    # gpsimd: cf32,A00x5,A01x2; scalar: B00x3; sync:
                    # cbf,rt00x2,rt01x2.  B01 follows at gtile 6.
                    disp_region(0, 0, "A", "gp", CH5[0:5])
                    disp_region(0, 1, "A", "gp", CH2)
                    disp_region(0, 0, "B", "sc", CH3[0:4])
                else:
                    # h1's B region: its buffer frees at image s-1's end
                    disp_region(s, 1, "B", "syb", CH2, extra_anchor=last["out"])
                if s + 1 < S:
                    disp_region(s + 1, 0, "A", "gp", CH2)
                ROUTER_G = 9 if s == 0 else -10  # stages 9-12, mixes from 13
                for h in (0, 1):
                    regA = regA_t[(s, h)]
                    regB = regB_t[(s, h)]
                    for t in range(27):
                        gtile = h * 27 + t
                        if s == 0:
                            if gtile == 2:
                                disp_rt(0, 0, "syb")
                            elif gtile == 3:
                                emit_reduce_chunk(0, 0, 0)
                                emit_reduce_chunk(0, 0, 1)
                            elif gtile == 4:
                                disp_rt(0, 1, "syb")
                                disp_region(0, 0, "B", "sc", CH3[3:5],
                                            extra_anchor=last["silu"])
                            elif gtile == 5:
                                emit_reduce_chunk(0, 1, 0)
                                emit_reduce_chunk(0, 1, 1)
                                disp_region(0, 0, "A", "syb", CH5[4:6])
                            elif gtile == 8:
                                disp_region(0, 0, "A", "syb", CH5[5:7])
                            elif gtile == 6:
                                disp_region(0, 1, "B", "sc", CH2,
                                            extra_anchor=last["silu"])
                        if s + 1 < S:
                            if gtile == 26:
                                disp_rt(s + 1, 0, "sc",
                                        extra_anchor=last["silu"])
                            elif gtile == 27:
                                disp_rt(s + 1, 1, "sc",
                                        extra_anchor=last["silu"])
                            elif gtile == 28:
                                disp_region(s + 1, 0, "B", "syb", CH2,
                                            extra_anchor=last["out"])
                            elif gtile == 30:
                                disp_region(s + 1, 1, "A", "sc", CH2,
                                            extra_anchor=last["silu"])
                            elif gtile in (38, 39):
                                emit_reduce_chunk(s + 1, 0, gtile - 38)
                            elif gtile in (44, 45):
                                emit_reduce_chunk(s + 1, 1, gtile - 44)
                            elif 48 <= gtile <= 51:
                                emit_router_stage(s + 1, gtile - 48)
                        N = NT if t < 26 else 324
                        n0 = t * NT
                        tsbs = []
                        for ep in range(2):
                            cps = pp.tile([128, N], F32, tag=f"pc{ep}", bufs=2)
                            base = ep * 640
                            nc.tensor.matmul(cps[:, :], cbf[:, base:base + 128],
                                             regA[:, n0:n0 + N], start=True, stop=False)
                            nc.tensor.matmul(cps[:, :], cbf[:, base + 128:base + 256],
                                             regA[:, n0 + 162:n0 + 162 + N],
                                             start=False, stop=False)
                            nc.tensor.matmul(cps[:, :], cbf[:, base + 256:base + 384],
                                             regA[:, n0 + 324:n0 + 324 + N],
                                             start=False, stop=False)
                            nc.tensor.matmul(cps[:, :], cbf[:, base + 384:base + 512],
                                             regB[:, n0:n0 + N], start=False, stop=False)
                            mlast = nc.tensor.matmul(
                                cps[:, :], cbf[:, base + 512:base + 640],
                                regB[:, n0 + 324:n0 + 324 + N],
                                start=False, stop=True)
                            tsb = wp.tile([128, N], BF16, tag=f"t{ep}", bufs=15)
                            slast = nc.scalar.activation(tsb[:, :], cps[:, :], AF.Silu,
                                                         bias=bnb_sb[:, ep:ep + 1])
                            tsbs.append(tsb)
                        last["conv"], last["silu"] = mlast, slast
                        pending.append((tsbs, N, h, t, s))
                        if 0 <= gtile - ROUTER_G <= 3:
                            emit_router_stage(s, gtile - ROUTER_G)
                        if s > 0 or gtile > ROUTER_G + 3:
                            k = 0
                            while len(pending) > 2 and k < 2:
                                emit_mix(pending.pop(0))
                                k += 1
                # drain this image's tail so bounce groups stay h-aligned
                while pending:
                    emit_mix(pending.pop(0))

    nc.compile()
    return nc


def _prep_weights(fc1_w, fc2_w, fc2_b, conv_w, bn_gamma, bn_beta, bn_mean, bn_var):
    scale = bn_gamma / np.sqrt(bn_var + BN_EPS)            # [E, C]
    shift = bn_beta - bn_mean * scale                      # [E, C]
    ws = conv_w * scale[:, :, None, None, None]            # [E, Co, Ci, 3, 3]

    # paired-tap lhsT blocks: [K=128 (2 taps x 64 ci), M=128 (2 experts x 64 co)]
    groups = [((0, 0), (0, 1)), ((1, 0), (1, 1)), ((2, 0), (2, 1)), ((0, 2), (1, 2))]
    wk = np.zeros((128, 1024), np.float32)
    for ep in range(2):
        for g, (ta, tb) in enumerate(groups):
            blk = np.stack([ws[:, :, :, ta[0], ta[1]], ws[:, :, :, tb[0], tb[1]]])
            blk = blk[:, 2 * ep:2 * ep + 2]                # [j, le, Co, Ci]
            lhsT = blk.transpose(0, 3, 1, 2).reshape(128, 128)
            wk[:, (ep * 4 + g) * 128:(ep * 4 + g + 1) * 128] = lhsT
    wks = np.zeros((128, 256), np.float32)                 # rows 64-127 stay zero
    s22 = ws[:, :, :, 2, 2]                                # [E, Co, Ci]
    for ep in range(2):
        blk = s22[2 * ep:2 * ep + 2]                       # [le, Co, Ci]
        wks[0:64, ep * 128:(ep + 1) * 128] = blk.transpose(2, 0, 1).reshape(64, 128)

    # packed bf16 consts: per-ep [wk 512 | wks 128] | fc1t2 | fc2tb
    cbf = np.zeros((128, 1300), np.float32)
    for ep in range(2):
        cbf[:, ep * 640:ep * 640 + 512] = wk[:, ep * 512:(ep + 1) * 512]
        cbf[:, ep * 640 + 512:ep * 640 + 640] = wks[:, ep * 128:(ep + 1) * 128]
    fc1t = fc1_w.T / float(H * W)                          # [64, 16]
    cbf[:, 1280:1296] = np.vstack([fc1t, fc1t])            # K=128 sums rt halves
    cbf[0:16, 1296:1300] = fc2_w.T                         # [16, 4]
    cbf[16, 1296:1300] = fc2_b                             # bias row (K=17)

    # packed f32 consts: idc | bnb | fc2b
    cf32 = np.zeros((128, 131), np.float32)
    cf32[:, 0:128] = np.concatenate([
        (np.arange(128)[:, None] % 64 == np.arange(64)[None, :]),
        np.zeros((128, 64), bool)], axis=1).astype(np.float32)
    cf32[:, 128] = np.concatenate([shift[0], shift[1]])
    cf32[:, 129] = np.concatenate([shift[2], shift[3]])
    cf32[0:4, 130] = fc2_b

    return {"cbf": cbf.astype(BF16_NP), "cf32": cf32}


def kernel(x, fc1_w, fc2_w, fc2_b, conv_w, bn_gamma, bn_beta, bn_mean, bn_var):
    from concourse.bass_utils import run_bass_kernel_spmd

    x = np.asarray(x, np.float32)
    reps = int(os.environ.get("BASS_KERNEL_REPS", "1"))
    key = f"nc{reps}"
    if key not in _CACHE:
        _CACHE[key] = _build_program(reps)
    nc = _CACHE[key]

    wmap = _prep_weights(np.asarray(fc1_w, np.float32), np.asarray(fc2_w, np.float32),
                         np.asarray(fc2_b, np.float32), np.asarray(conv_w, np.float32),
                         np.asarray(bn_gamma, np.float32), np.asarray(bn_beta, np.float32),
                         np.asarray(bn_mean, np.float32), np.asarray(bn_var, np.float32))

    # host-side zero-pad + bf16 cast into the flat 162x162 (+margin) layout,
    # then 4 tap-shifted copies (shifts 0/1/2/164) so each SBUF region loads
    # as one full-width 128-partition DMA
    xp = np.zeros((B, 4, C, LHOST), BF16_NP)
    xpad = xp[:, 0, :, :LP].reshape(B, C, H + 2, WP)
    xpad[:, :, 1:H + 1, 1:W + 1] = x.astype(BF16_NP)
    for j, sft in ((1, 1), (2, 2), (3, 164)):
        xp[:, j, :, :LHOST - sft] = xp[:, 0, :, sft:]
    # fp8 copy for the router mean-pool (noise ~1e-4 of the logits)
    xq = np.zeros((B, C, LHOST), FP8_NP)
    xq[:, :, :LP] = xp[:, 0, :, :LP].astype(FP8_NP)

    in_maps = []
    for c in range(NCORES):
        m = dict(wmap)
        m["xp"] = xp[c * IMG_PER_CORE:(c + 1) * IMG_PER_CORE]
        m["xq"] = xq[c * IMG_PER_CORE:(c + 1) * IMG_PER_CORE]
        in_maps.append(m)

    trace = bool(int(os.environ.get("BASS_KERNEL_TRACE", "0")))
    res = run_bass_kernel_spmd(nc, in_maps, list(range(NCORES)), trace=trace)
    _CACHE["last_results"] = res
    return np.concatenate([res.results[c]["out"] for c in range(NCORES)], axis=0)
